# revision 17
# baseline (speedup 1.0000x reference)
"""MoE ExpertAllocation router kernel for Trainium2 (8 NeuronCores, Bass/Tile).

Reference math (B=8, S=2048, D=4096, E=64):
  logits       = x @ W + b                         [B,S,E]
  router_probs = softmax(logits, -1)
  top_idx      = top_k(router_probs, 2).indices    [B,S,2] int32
  f_i          = one-hot-top2 counts / B
  P_i          = router_probs.sum((0,1)) / B
  aux_loss     = 0.01 * E * sum(f_i * P_i)
  capacity mask: buffer_size = (B/E)*1.25 = 0.15625 < 1, and the inclusive
  cumsum of the 0/1 routed_experts is >= 1 wherever routed_experts == 1, so
  expert_mask zeroes every routed entry:
    routed_experts (returned) == 0 and routed_probs == 0 identically.

Device work per core (tokens sharded 8 x 2048):
  - fp32 GEMM, W_k [128,64] stationary, x^T streamed; two k-chains run
    concurrently on PE column halves h0/h1 (2x col tiling), partial halves
    summed during PSUM evacuation
  - GEMM runs in two half-token phases so phase-1 softmax/top-2 stats
    overlap phase-2 GEMM DMA/compute
  - PE transpose of logits^T -> [128 tok, 64 expert] tiles
  - DVE max8/max_index -> top-2 indices (tie order matches jax top_k)
  - ACT exp(x - max) with fused row-sum; DVE reciprocal
  - P_i partial = sum_t exp[t,e] * recip[t] via tiny PE matmuls
Host: shards/transposes x, packs W into [128, KT*E], reduces P_i partials,
bincounts f_i, assembles outputs.
"""

import os
import sys

import numpy as np

for _p in ("/opt/trn_rl_repo", os.path.expanduser("~/.axon_site/_ro/trn_rl_repo")):
    if os.path.isdir(_p) and _p not in sys.path:
        sys.path.append(_p)

import concourse.bass as bass  # noqa: E402
import concourse.tile as tile  # noqa: E402
from concourse import bacc, mybir  # noqa: E402
from concourse import bass_utils  # noqa: E402
from concourse.masks import make_identity  # noqa: E402

B, S, D, E = 8, 2048, 4096, 64
N_CORES = 8
T = (B * S) // N_CORES  # tokens per core = 2048
P = 128
KT = D // P  # 32 k-tiles
NT = T // P  # 16 token tiles of 128
GF = 512  # GEMM moving free dim (fp32 max, one PSUM bank)
TB = T // GF  # 4 token blocks
# Asymmetric phases (in 512-token GEMM blocks): the big first phase keeps
# DMA chunks large; the small last phase leaves only a short stats tail.
PHASE_TBS = [3, 1]
PHASE_TB0 = [0, 3]  # prefix sums

F32 = mybir.dt.float32
U32 = mybir.dt.uint32


def _build_kernel_body(ctx, tc, xt, w, b, top_idx, p_part):
    nc = tc.nc

    const_pool = ctx.enter_context(tc.tile_pool(name="const", bufs=1))
    xpool = ctx.enter_context(tc.tile_pool(name="xtiles", bufs=10))
    gpsum = ctx.enter_context(tc.tile_pool(name="gpsum", bufs=TB, space="PSUM"))
    tpsum = ctx.enter_context(tc.tile_pool(name="tpsum", bufs=2, space="PSUM"))
    ppsum = ctx.enter_context(tc.tile_pool(name="ppsum", bufs=1, space="PSUM"))
    work = ctx.enter_context(tc.tile_pool(name="work", bufs=1))
    small = ctx.enter_context(tc.tile_pool(name="small", bufs=4))

    # W is host-packed to [128, KT*E] (w_packed[p, k*E+e] = W[k*128+p, e]) so
    # it loads as fully-contiguous DMAs; it rides the ScalarE HWDGE ring
    # (qActDynamicHW) so the Sync ring streams x^T from the first trigger.
    # Two halves so the first matmuls only wait on the low-k half.
    KH = KT // 2
    w_lo = const_pool.tile([P, KH, E], F32)
    nc.scalar.dma_start(
        w_lo, w[:, : KH * E].rearrange("p (ko e) -> p ko e", e=E)
    )
    w_hi = const_pool.tile([P, KH, E], F32)
    nc.scalar.dma_start(
        w_hi, w[:, KH * E :].rearrange("p (ko e) -> p ko e", e=E)
    )

    def wk(k):
        return w_lo[:, k, :] if k < KH else w_hi[:, k - KH, :]

    b_sb = const_pool.tile([E, 1], F32)
    nc.scalar.dma_start(b_sb, b[:, None])
    ident = const_pool.tile([E, E], F32)
    make_identity(nc, ident)

    # PE warmup: ~5us of dummy matmuls so HAM reaches K=8/8 before the real
    # GEMM starts (cold first matmuls at 1.2 GHz ripple backpressure into
    # the DMA pipeline).  ident is the only dependency; results are never
    # read.
    wm_ps = ppsum.tile([E, E], F32, name="warm_ps")
    for _ in range(24):
        nc.tensor.matmul(wm_ps, lhsT=ident, rhs=ident, start=True, stop=True)

    # Persistent work tiles
    logitsT = work.tile([E, T], F32)
    logits3 = work.tile([P, NT, E], F32)
    ex3 = work.tile([P, NT, E], F32)
    rec = work.tile([P, NT], F32)
    idxacc = work.tile([P, NT, 2], U32)

    gps = [
        gpsum.tile([P, GF], F32, tag="gps", name=f"gps{i}") for i in range(TB)
    ]
    pp = ppsum.tile([1, E], F32)
    KP = KT // 2

    def gemm_phase(ph):
        tbs, tb0 = PHASE_TBS[ph], PHASE_TB0[ph]
        c0 = tb0 * GF  # column (token) offset of this phase
        tph = tbs * GF
        for j in range(KP):
            k0, k1 = 2 * j, 2 * j + 1
            # The two x^T streams ride the SP HWDGE ring and the GpSimd
            # SWDGE queue: one ring alone caps at ~310 GB/s, and neither
            # engine appears in the stats path, so a phase's stats can
            # never block the next phase's streaming (in-order engines).
            xt_e = xpool.tile([P, tph], F32, tag=f"xt{ph}", name=f"xte{ph}_{j}")
            nc.sync.dma_start(xt_e, xt[k0 * P : (k0 + 1) * P, c0 : c0 + tph])
            xt_o = xpool.tile([P, tph], F32, tag=f"xt{ph}", name=f"xto{ph}_{j}")
            nc.gpsimd.dma_start(xt_o, xt[k1 * P : (k1 + 1) * P, c0 : c0 + tph])
            for tb in range(tbs):
                g = gps[tb0 + tb]
                nc.tensor.matmul(
                    g[0:E, :],
                    lhsT=wk(k0),
                    rhs=xt_e[:, tb * GF : (tb + 1) * GF],
                    start=(j == 0),
                    stop=(j == KP - 1),
                    tile_position=(0, 0),
                    skip_group_check=True,
                )
                nc.tensor.matmul(
                    g[E : 2 * E, :],
                    lhsT=wk(k1),
                    rhs=xt_o[:, tb * GF : (tb + 1) * GF],
                    start=(j == 0),
                    stop=(j == KP - 1),
                    tile_position=(0, E),
                    skip_group_check=True,
                )

    def stats_phase(ph):
        tbs, tb0 = PHASE_TBS[ph], PHASE_TB0[ph]
        ntp = tbs * GF // P  # 128-token tiles in this phase
        nt0 = tb0 * GF // P
        # PSUM -> SBUF with bias add + h0/h1 combine (DVE reads one PSUM
        # operand per op)
        for tb in range(tbs):
            g = gps[tb0 + tb]
            seg = logitsT[:, (tb0 + tb) * GF : (tb0 + tb + 1) * GF]
            nc.vector.tensor_scalar(
                out=seg,
                in0=g[0:E, :],
                scalar1=b_sb,
                scalar2=None,
                op0=mybir.AluOpType.add,
            )
            nc.vector.tensor_add(out=seg, in0=seg, in1=g[E : 2 * E, :])

        for ti in range(ntp):
            t = nt0 + ti
            tp = tpsum.tile([P, E], F32, tag="tp")
            nc.tensor.transpose(tp, logitsT[:, t * P : (t + 1) * P], ident)
            # Alternate the PSUM evacuation copy between ScalarE and DVE to
            # balance the per-tile pipeline across engines.
            if ti % 2 == 0:
                nc.scalar.copy(out=logits3[:, t, :], in_=tp)
            else:
                nc.vector.tensor_copy(out=logits3[:, t, :], in_=tp)

        for ti in range(ntp):
            t = nt0 + ti
            lg = logits3[:, t, :]
            mx8 = small.tile([P, 8], F32, tag="mx8")
            nc.vector.max(out=mx8, in_=lg)
            ix8 = small.tile([P, 8], U32, tag="ix8")
            nc.vector.max_index(out=ix8, in_max=mx8, in_values=lg)
            nc.vector.tensor_copy(out=idxacc[:, t, :], in_=ix8[:, 0:2])
            negmx = small.tile([P, 1], F32, tag="negmx")
            nc.vector.tensor_scalar_mul(negmx, mx8[:, 0:1], -1.0)
            ssum = small.tile([P, 1], F32, tag="ssum")
            nc.scalar.activation(
                out=ex3[:, t, :],
                in_=lg,
                func=mybir.ActivationFunctionType.Exp,
                bias=negmx,
                scale=1.0,
                accum_out=ssum,
            )
            nc.vector.reciprocal(out=rec[:, t : t + 1], in_=ssum)

        # Emit this phase's top-2 indices: SBUF [p, i, j] -> DRAM [(i p), j]
        # on the ACT ring (the Sync/GpSimd streams must never wait on stats)
        nc.scalar.dma_start(
            top_idx[nt0 * P : (nt0 + ntp) * P, :].rearrange(
                "(i p) j -> p i j", p=P
            ),
            idxacc[:, nt0 : nt0 + ntp, :],
        )

    for ph in range(len(PHASE_TBS)):
        gemm_phase(ph)
        stats_phase(ph)

    # P_i partials: one PSUM row accumulated by tiny PE matmuls at the end
    # (these wait on the full stats chain, so they must sit after all GEMM
    # matmuls in PE program order)
    for t in range(NT):
        nc.tensor.matmul(
            pp,
            lhsT=rec[:, t : t + 1],
            rhs=ex3[:, t, :],
            start=(t == 0),
            stop=(t == NT - 1),
            skip_group_check=True,
        )

    p_sb = small.tile([1, E], F32, tag="pout")
    nc.vector.tensor_copy(out=p_sb, in_=pp)
    nc.scalar.dma_start(p_part, p_sb)


_COMPILED_NC = None


def _get_compiled():
    global _COMPILED_NC
    if _COMPILED_NC is not None:
        return _COMPILED_NC
    from contextlib import ExitStack

    nc = bacc.Bacc(
        "TRN2",
        target_bir_lowering=False,
        debug=False,
        enable_asserts=False,
        num_devices=N_CORES,
    )
    xt = nc.dram_tensor("xt", [D, T], F32, kind="ExternalInput").ap()
    w = nc.dram_tensor("w", [P, KT * E], F32, kind="ExternalInput").ap()
    b = nc.dram_tensor("b", [E], F32, kind="ExternalInput").ap()
    top_idx = nc.dram_tensor("top_idx", [T, 2], U32, kind="ExternalOutput").ap()
    p_part = nc.dram_tensor("p_part", [1, E], F32, kind="ExternalOutput").ap()

    with tile.TileContext(nc) as tc:
        with ExitStack() as ctx:
            _build_kernel_body(ctx, tc, xt, w, b, top_idx, p_part)
    nc.compile()
    _COMPILED_NC = nc
    return nc


def _run_device(x, W, b, trace=False):
    nc = _get_compiled()
    xf = np.ascontiguousarray(np.asarray(x, dtype=np.float32)).reshape(B * S, D)
    Wf = np.asarray(W, dtype=np.float32)
    # w_packed[p, k*E+e] = W[k*128+p, e]
    w_packed = np.ascontiguousarray(
        Wf.reshape(KT, P, E).transpose(1, 0, 2).reshape(P, KT * E)
    )
    bf = np.ascontiguousarray(np.asarray(b, dtype=np.float32))
    in_maps = []
    for c in range(N_CORES):
        shard = np.ascontiguousarray(xf[c * T : (c + 1) * T, :].T)
        in_maps.append({"xt": shard, "w": w_packed, "b": bf})
    res = bass_utils.run_bass_kernel_spmd(
        nc, in_maps, core_ids=list(range(N_CORES)), trace=trace
    )
    return res


def kernel(x, W, b):
    res = _run_device(x, W, b, trace=False)
    return _assemble(res.results)


def _assemble(results):
    top_u = np.concatenate(
        [results[c]["top_idx"] for c in range(N_CORES)], axis=0
    )  # [B*S, 2] uint32
    top_idx = top_u.astype(np.int32).reshape(B, S, 2)

    p_sum = np.zeros(E, dtype=np.float64)
    for c in range(N_CORES):
        p_sum += results[c]["p_part"][0].astype(np.float64)
    P_i = p_sum / float(B)
    f_i = np.bincount(top_u.ravel().astype(np.int64), minlength=E).astype(
        np.float64
    ) / float(B)
    aux_loss = np.float32(0.01 * E * np.sum(f_i * P_i))

    routed_experts = np.zeros((B, S, E), dtype=np.float32)
    routed_probs = np.zeros((B, S, E), dtype=np.float32)
    return routed_experts, routed_probs, top_idx, aux_loss


# revision 18
# speedup vs baseline: 1.0259x; 1.0259x over previous
"""MoE ExpertAllocation router kernel for Trainium2 (8 NeuronCores, Bass/Tile).

Reference math (B=8, S=2048, D=4096, E=64):
  logits       = x @ W + b                         [B,S,E]
  router_probs = softmax(logits, -1)
  top_idx      = top_k(router_probs, 2).indices    [B,S,2] int32
  f_i          = one-hot-top2 counts / B
  P_i          = router_probs.sum((0,1)) / B
  aux_loss     = 0.01 * E * sum(f_i * P_i)
  capacity mask: buffer_size = (B/E)*1.25 = 0.15625 < 1, and the inclusive
  cumsum of the 0/1 routed_experts is >= 1 wherever routed_experts == 1, so
  expert_mask zeroes every routed entry:
    routed_experts (returned) == 0 and routed_probs == 0 identically.

Device work per core (tokens sharded 8 x 2048):
  - fp32 GEMM, W_k [128,64] stationary, x^T streamed; two k-chains run
    concurrently on PE column halves h0/h1 (2x col tiling), partial halves
    summed during PSUM evacuation
  - GEMM runs in two half-token phases so phase-1 softmax/top-2 stats
    overlap phase-2 GEMM DMA/compute
  - PE transpose of logits^T -> [128 tok, 64 expert] tiles
  - DVE max8/max_index -> top-2 indices (tie order matches jax top_k)
  - ACT exp(x - max) with fused row-sum; DVE reciprocal
  - P_i partial = sum_t exp[t,e] * recip[t] via tiny PE matmuls
Host: shards/transposes x, packs W into [128, KT*E], reduces P_i partials,
bincounts f_i, assembles outputs.
"""

import os
import sys

import numpy as np

for _p in ("/opt/trn_rl_repo", os.path.expanduser("~/.axon_site/_ro/trn_rl_repo")):
    if os.path.isdir(_p) and _p not in sys.path:
        sys.path.append(_p)

import concourse.bass as bass  # noqa: E402
import concourse.tile as tile  # noqa: E402
from concourse import bacc, mybir  # noqa: E402
from concourse import bass_utils  # noqa: E402
from concourse.masks import make_identity  # noqa: E402

B, S, D, E = 8, 2048, 4096, 64
N_CORES = 8
T = (B * S) // N_CORES  # tokens per core = 2048
P = 128
KT = D // P  # 32 k-tiles
NT = T // P  # 16 token tiles of 128
GF = 512  # GEMM moving free dim (fp32 max, one PSUM bank)
TB = T // GF  # 4 token blocks
# Asymmetric phases (in 512-token GEMM blocks): the big first phase keeps
# DMA chunks large; the small last phase leaves only a short stats tail.
PHASE_TBS = [3, 1]
PHASE_TB0 = [0, 3]  # prefix sums
MAXTPH = max(PHASE_TBS) * GF

F32 = mybir.dt.float32
U32 = mybir.dt.uint32


def _build_kernel_body(ctx, tc, xt, w, b, top_idx, p_part):
    nc = tc.nc

    const_pool = ctx.enter_context(tc.tile_pool(name="const", bufs=1))
    xpool = ctx.enter_context(tc.tile_pool(name="xtiles", bufs=10))
    gpsum = ctx.enter_context(tc.tile_pool(name="gpsum", bufs=TB, space="PSUM"))
    tpsum = ctx.enter_context(tc.tile_pool(name="tpsum", bufs=2, space="PSUM"))
    ppsum = ctx.enter_context(tc.tile_pool(name="ppsum", bufs=1, space="PSUM"))
    work = ctx.enter_context(tc.tile_pool(name="work", bufs=1))
    small = ctx.enter_context(tc.tile_pool(name="small", bufs=4))

    # W is host-packed to [128, KT*E] (w_packed[p, k*E+e] = W[k*128+p, e]) so
    # it loads as fully-contiguous DMAs; it rides the ScalarE HWDGE ring
    # (qActDynamicHW) so the Sync ring streams x^T from the first trigger.
    # Two halves so the first matmuls only wait on the low-k half.
    KH = KT // 2
    w_lo = const_pool.tile([P, KH, E], F32)
    nc.scalar.dma_start(
        w_lo, w[:, : KH * E].rearrange("p (ko e) -> p ko e", e=E)
    )
    w_hi = const_pool.tile([P, KH, E], F32)
    nc.scalar.dma_start(
        w_hi, w[:, KH * E :].rearrange("p (ko e) -> p ko e", e=E)
    )

    def wk(k):
        return w_lo[:, k, :] if k < KH else w_hi[:, k - KH, :]

    b_sb = const_pool.tile([E, 1], F32)
    nc.scalar.dma_start(b_sb, b[:, None])
    ident = const_pool.tile([E, E], F32)
    make_identity(nc, ident)

    # PE warmup: ~5us of dummy matmuls so HAM reaches K=8/8 before the real
    # GEMM starts (cold first matmuls at 1.2 GHz ripple backpressure into
    # the DMA pipeline).  ident is the only dependency; results are never
    # read.
    wm_ps = ppsum.tile([E, E], F32, name="warm_ps")
    for _ in range(24):
        nc.tensor.matmul(wm_ps, lhsT=ident, rhs=ident, start=True, stop=True)

    # Persistent work tiles
    logitsT = work.tile([E, T], F32)
    logits3 = work.tile([P, NT, E], F32)
    ex3 = work.tile([P, NT, E], F32)
    rec = work.tile([P, NT], F32)
    idxacc = work.tile([P, NT, 2], U32)

    gps = [
        gpsum.tile([P, GF], F32, tag="gps", name=f"gps{i}") for i in range(TB)
    ]
    pp = ppsum.tile([1, E], F32)
    KP = KT // 2

    def gemm_phase(ph):
        tbs, tb0 = PHASE_TBS[ph], PHASE_TB0[ph]
        c0 = tb0 * GF  # column (token) offset of this phase
        tph = tbs * GF
        for j in range(KP):
            k0, k1 = 2 * j, 2 * j + 1
            # The two x^T streams ride the SP HWDGE ring and the GpSimd
            # SWDGE queue: one ring alone caps at ~310 GB/s, and neither
            # engine appears in the stats path, so a phase's stats can
            # never block the next phase's streaming (in-order engines).
            # Uniform slot shape + one tag across phases: slots recycle
            # round-robin at pool depth; mixed sizes would make a phase's
            # first tiles WAR-wait on the previous phase's whole GEMM.
            xt_e = xpool.tile([P, MAXTPH], F32, tag="xt", name=f"xte{ph}_{j}")[
                :, :tph
            ]
            nc.sync.dma_start(xt_e, xt[k0 * P : (k0 + 1) * P, c0 : c0 + tph])
            xt_o = xpool.tile([P, MAXTPH], F32, tag="xt", name=f"xto{ph}_{j}")[
                :, :tph
            ]
            nc.gpsimd.dma_start(xt_o, xt[k1 * P : (k1 + 1) * P, c0 : c0 + tph])
            for tb in range(tbs):
                g = gps[tb0 + tb]
                nc.tensor.matmul(
                    g[0:E, :],
                    lhsT=wk(k0),
                    rhs=xt_e[:, tb * GF : (tb + 1) * GF],
                    start=(j == 0),
                    stop=(j == KP - 1),
                    tile_position=(0, 0),
                    skip_group_check=True,
                )
                nc.tensor.matmul(
                    g[E : 2 * E, :],
                    lhsT=wk(k1),
                    rhs=xt_o[:, tb * GF : (tb + 1) * GF],
                    start=(j == 0),
                    stop=(j == KP - 1),
                    tile_position=(0, E),
                    skip_group_check=True,
                )

    def stats_phase(ph):
        tbs, tb0 = PHASE_TBS[ph], PHASE_TB0[ph]
        ntp = tbs * GF // P  # 128-token tiles in this phase
        nt0 = tb0 * GF // P
        # PSUM -> SBUF with bias add + h0/h1 combine (DVE reads one PSUM
        # operand per op)
        for tb in range(tbs):
            g = gps[tb0 + tb]
            seg = logitsT[:, (tb0 + tb) * GF : (tb0 + tb + 1) * GF]
            nc.vector.tensor_scalar(
                out=seg,
                in0=g[0:E, :],
                scalar1=b_sb,
                scalar2=None,
                op0=mybir.AluOpType.add,
            )
            nc.vector.tensor_add(out=seg, in0=seg, in1=g[E : 2 * E, :])

        for ti in range(ntp):
            t = nt0 + ti
            tp = tpsum.tile([P, E], F32, tag="tp")
            nc.tensor.transpose(tp, logitsT[:, t * P : (t + 1) * P], ident)
            # Alternate the PSUM evacuation copy between ScalarE and DVE to
            # balance the per-tile pipeline across engines.
            if ti % 2 == 0:
                nc.scalar.copy(out=logits3[:, t, :], in_=tp)
            else:
                nc.vector.tensor_copy(out=logits3[:, t, :], in_=tp)

        for ti in range(ntp):
            t = nt0 + ti
            lg = logits3[:, t, :]
            mx8 = small.tile([P, 8], F32, tag="mx8")
            nc.vector.max(out=mx8, in_=lg)
            ix8 = small.tile([P, 8], U32, tag="ix8")
            nc.vector.max_index(out=ix8, in_max=mx8, in_values=lg)
            nc.vector.tensor_copy(out=idxacc[:, t, :], in_=ix8[:, 0:2])
            negmx = small.tile([P, 1], F32, tag="negmx")
            nc.vector.tensor_scalar_mul(negmx, mx8[:, 0:1], -1.0)
            ssum = small.tile([P, 1], F32, tag="ssum")
            nc.scalar.activation(
                out=ex3[:, t, :],
                in_=lg,
                func=mybir.ActivationFunctionType.Exp,
                bias=negmx,
                scale=1.0,
                accum_out=ssum,
            )
            nc.vector.reciprocal(out=rec[:, t : t + 1], in_=ssum)

        # Emit this phase's top-2 indices: SBUF [p, i, j] -> DRAM [(i p), j]
        # on the ACT ring (the Sync/GpSimd streams must never wait on stats)
        nc.scalar.dma_start(
            top_idx[nt0 * P : (nt0 + ntp) * P, :].rearrange(
                "(i p) j -> p i j", p=P
            ),
            idxacc[:, nt0 : nt0 + ntp, :],
        )

    for ph in range(len(PHASE_TBS)):
        gemm_phase(ph)
        stats_phase(ph)

    # P_i partials: one PSUM row accumulated by tiny PE matmuls at the end
    # (these wait on the full stats chain, so they must sit after all GEMM
    # matmuls in PE program order)
    for t in range(NT):
        nc.tensor.matmul(
            pp,
            lhsT=rec[:, t : t + 1],
            rhs=ex3[:, t, :],
            start=(t == 0),
            stop=(t == NT - 1),
            skip_group_check=True,
        )

    p_sb = small.tile([1, E], F32, tag="pout")
    nc.vector.tensor_copy(out=p_sb, in_=pp)
    nc.scalar.dma_start(p_part, p_sb)


_COMPILED_NC = None


def _get_compiled():
    global _COMPILED_NC
    if _COMPILED_NC is not None:
        return _COMPILED_NC
    from contextlib import ExitStack

    nc = bacc.Bacc(
        "TRN2",
        target_bir_lowering=False,
        debug=False,
        enable_asserts=False,
        num_devices=N_CORES,
    )
    xt = nc.dram_tensor("xt", [D, T], F32, kind="ExternalInput").ap()
    w = nc.dram_tensor("w", [P, KT * E], F32, kind="ExternalInput").ap()
    b = nc.dram_tensor("b", [E], F32, kind="ExternalInput").ap()
    top_idx = nc.dram_tensor("top_idx", [T, 2], U32, kind="ExternalOutput").ap()
    p_part = nc.dram_tensor("p_part", [1, E], F32, kind="ExternalOutput").ap()

    with tile.TileContext(nc) as tc:
        with ExitStack() as ctx:
            _build_kernel_body(ctx, tc, xt, w, b, top_idx, p_part)
    nc.compile()
    _COMPILED_NC = nc
    return nc


def _run_device(x, W, b, trace=False):
    nc = _get_compiled()
    xf = np.ascontiguousarray(np.asarray(x, dtype=np.float32)).reshape(B * S, D)
    Wf = np.asarray(W, dtype=np.float32)
    # w_packed[p, k*E+e] = W[k*128+p, e]
    w_packed = np.ascontiguousarray(
        Wf.reshape(KT, P, E).transpose(1, 0, 2).reshape(P, KT * E)
    )
    bf = np.ascontiguousarray(np.asarray(b, dtype=np.float32))
    in_maps = []
    for c in range(N_CORES):
        shard = np.ascontiguousarray(xf[c * T : (c + 1) * T, :].T)
        in_maps.append({"xt": shard, "w": w_packed, "b": bf})
    res = bass_utils.run_bass_kernel_spmd(
        nc, in_maps, core_ids=list(range(N_CORES)), trace=trace
    )
    return res


def kernel(x, W, b):
    res = _run_device(x, W, b, trace=False)
    return _assemble(res.results)


def _assemble(results):
    top_u = np.concatenate(
        [results[c]["top_idx"] for c in range(N_CORES)], axis=0
    )  # [B*S, 2] uint32
    top_idx = top_u.astype(np.int32).reshape(B, S, 2)

    p_sum = np.zeros(E, dtype=np.float64)
    for c in range(N_CORES):
        p_sum += results[c]["p_part"][0].astype(np.float64)
    P_i = p_sum / float(B)
    f_i = np.bincount(top_u.ravel().astype(np.int64), minlength=E).astype(
        np.float64
    ) / float(B)
    aux_loss = np.float32(0.01 * E * np.sum(f_i * P_i))

    routed_experts = np.zeros((B, S, E), dtype=np.float32)
    routed_probs = np.zeros((B, S, E), dtype=np.float32)
    return routed_experts, routed_probs, top_idx, aux_loss


# revision 20
# speedup vs baseline: 1.2313x; 1.2002x over previous
"""MoE ExpertAllocation router kernel for Trainium2 (8 NeuronCores, Bass/Tile).

Reference math (B=8, S=2048, D=4096, E=64):
  logits       = x @ W + b                         [B,S,E]
  router_probs = softmax(logits, -1)
  top_idx      = top_k(router_probs, 2).indices    [B,S,2] int32
  f_i          = one-hot-top2 counts / B
  P_i          = router_probs.sum((0,1)) / B
  aux_loss     = 0.01 * E * sum(f_i * P_i)
  capacity mask: buffer_size = (B/E)*1.25 = 0.15625 < 1, and the inclusive
  cumsum of the 0/1 routed_experts is >= 1 wherever routed_experts == 1, so
  expert_mask zeroes every routed entry:
    routed_experts (returned) == 0 and routed_probs == 0 identically.

Device work per core (tokens sharded 8 x 2048):
  - fp32 GEMM, W_k [128,64] stationary, x^T streamed; two k-chains run
    concurrently on PE column halves h0/h1 (2x col tiling), partial halves
    summed during PSUM evacuation
  - GEMM runs in two half-token phases so phase-1 softmax/top-2 stats
    overlap phase-2 GEMM DMA/compute
  - PE transpose of logits^T -> [128 tok, 64 expert] tiles
  - DVE max8/max_index -> top-2 indices (tie order matches jax top_k)
  - ACT exp(x - max) with fused row-sum; DVE reciprocal
  - P_i partial = sum_t exp[t,e] * recip[t] via tiny PE matmuls
Host: shards/transposes x, packs W into [128, KT*E], reduces P_i partials,
bincounts f_i, assembles outputs.
"""

import os
import sys

import numpy as np

for _p in ("/opt/trn_rl_repo", os.path.expanduser("~/.axon_site/_ro/trn_rl_repo")):
    if os.path.isdir(_p) and _p not in sys.path:
        sys.path.append(_p)

import concourse.bass as bass  # noqa: E402
import concourse.tile as tile  # noqa: E402
from concourse import bacc, mybir  # noqa: E402
from concourse import bass_utils  # noqa: E402
from concourse.masks import make_identity  # noqa: E402

B, S, D, E = 8, 2048, 4096, 64
N_CORES = 8
T = (B * S) // N_CORES  # tokens per core = 2048
P = 128
KT = D // P  # 32 k-tiles
NT = T // P  # 16 token tiles of 128
GF = 512  # GEMM moving free dim (fp32 max, one PSUM bank)
TB = T // GF  # 4 token blocks
# Asymmetric phases (in 512-token GEMM blocks): the big first phase keeps
# DMA chunks large; the small last phase leaves only a short stats tail.
PHASE_TBS = [3, 1]
PHASE_TB0 = [0, 3]  # prefix sums
MAXTPH = max(PHASE_TBS) * GF

F32 = mybir.dt.float32
U32 = mybir.dt.uint32


def _build_kernel_body(ctx, tc, xt, w, b, top_idx, p_part):
    nc = tc.nc

    const_pool = ctx.enter_context(tc.tile_pool(name="const", bufs=1))
    xpool = ctx.enter_context(tc.tile_pool(name="xtiles", bufs=10))
    gpsum = ctx.enter_context(tc.tile_pool(name="gpsum", bufs=TB, space="PSUM"))
    tpsum = ctx.enter_context(tc.tile_pool(name="tpsum", bufs=2, space="PSUM"))
    ppsum = ctx.enter_context(tc.tile_pool(name="ppsum", bufs=1, space="PSUM"))
    work = ctx.enter_context(tc.tile_pool(name="work", bufs=1))
    small = ctx.enter_context(tc.tile_pool(name="small", bufs=4))

    # W is host-packed to [128, KT*E] (w_packed[p, k*E+e] = W[k*128+p, e]) so
    # it loads as fully-contiguous DMAs; it rides the ScalarE HWDGE ring
    # (qActDynamicHW) so the Sync ring streams x^T from the first trigger.
    # Two halves so the first matmuls only wait on the low-k half.
    KH = KT // 2
    w_lo = const_pool.tile([P, KH, E], F32)
    nc.scalar.dma_start(
        w_lo, w[:, : KH * E].rearrange("p (ko e) -> p ko e", e=E)
    )
    w_hi = const_pool.tile([P, KH, E], F32)
    nc.scalar.dma_start(
        w_hi, w[:, KH * E :].rearrange("p (ko e) -> p ko e", e=E)
    )

    def wk(k):
        return w_lo[:, k, :] if k < KH else w_hi[:, k - KH, :]

    b_sb = const_pool.tile([E, 1], F32)
    nc.scalar.dma_start(b_sb, b[:, None])
    ident = const_pool.tile([E, E], F32)
    make_identity(nc, ident)

    # PE warmup: ~5us of dummy matmuls so HAM reaches K=8/8 before the real
    # GEMM starts (cold first matmuls at 1.2 GHz ripple backpressure into
    # the DMA pipeline).  ident is the only dependency; results are never
    # read.
    wm_ps = ppsum.tile([E, E], F32, name="warm_ps")
    for _ in range(24):
        nc.tensor.matmul(wm_ps, lhsT=ident, rhs=ident, start=True, stop=True)

    # Persistent work tiles
    logitsT = work.tile([E, T], F32)
    logits3 = work.tile([P, NT, E], F32)
    ex3 = work.tile([P, NT, E], F32)
    rec = work.tile([P, NT], F32)
    idxacc = work.tile([P, NT, 2], U32)

    gps = [
        gpsum.tile([P, GF], F32, tag="gps", name=f"gps{i}") for i in range(TB)
    ]
    pp = ppsum.tile([1, E], F32)
    KP = KT // 2

    def gemm_phase(ph):
        tbs, tb0 = PHASE_TBS[ph], PHASE_TB0[ph]
        c0 = tb0 * GF  # column (token) offset of this phase
        tph = tbs * GF
        for j in range(KP):
            k0, k1 = 2 * j, 2 * j + 1
            # The two x^T streams ride the SP and ACT HWDGE rings (one
            # ring alone caps at ~310 GB/s; SWDGE is far slower).  The
            # ACT ring also serves stats exps, so stats are split into
            # pre/post and post-stats are emitted after the next phase's
            # triggers (engines are in-order).
            # Uniform slot shape + one tag across phases: slots recycle
            # round-robin at pool depth; mixed sizes would make a phase's
            # first tiles WAR-wait on the previous phase's whole GEMM.
            xt_e = xpool.tile([P, MAXTPH], F32, tag="xt", name=f"xte{ph}_{j}")[
                :, :tph
            ]
            nc.sync.dma_start(xt_e, xt[k0 * P : (k0 + 1) * P, c0 : c0 + tph])
            xt_o = xpool.tile([P, MAXTPH], F32, tag="xt", name=f"xto{ph}_{j}")[
                :, :tph
            ]
            nc.scalar.dma_start(xt_o, xt[k1 * P : (k1 + 1) * P, c0 : c0 + tph])
            for tb in range(tbs):
                g = gps[tb0 + tb]
                nc.tensor.matmul(
                    g[0:E, :],
                    lhsT=wk(k0),
                    rhs=xt_e[:, tb * GF : (tb + 1) * GF],
                    start=(j == 0),
                    stop=(j == KP - 1),
                    tile_position=(0, 0),
                    skip_group_check=True,
                )
                nc.tensor.matmul(
                    g[E : 2 * E, :],
                    lhsT=wk(k1),
                    rhs=xt_o[:, tb * GF : (tb + 1) * GF],
                    start=(j == 0),
                    stop=(j == KP - 1),
                    tile_position=(0, E),
                    skip_group_check=True,
                )

    negmx3 = work.tile([P, NT], F32)

    def stats_pre(ph):
        # DVE/PE-only part of the stats: runs under the next phase's GEMM
        # without ever sitting ahead of its DMA triggers on Sync/ACT.
        tbs, tb0 = PHASE_TBS[ph], PHASE_TB0[ph]
        ntp = tbs * GF // P  # 128-token tiles in this phase
        nt0 = tb0 * GF // P
        # PSUM -> SBUF with bias add + h0/h1 combine (DVE reads one PSUM
        # operand per op)
        for tb in range(tbs):
            g = gps[tb0 + tb]
            seg = logitsT[:, (tb0 + tb) * GF : (tb0 + tb + 1) * GF]
            nc.vector.tensor_scalar(
                out=seg,
                in0=g[0:E, :],
                scalar1=b_sb,
                scalar2=None,
                op0=mybir.AluOpType.add,
            )
            nc.vector.tensor_add(out=seg, in0=seg, in1=g[E : 2 * E, :])

        for ti in range(ntp):
            t = nt0 + ti
            tp = tpsum.tile([P, E], F32, tag="tp")
            nc.tensor.transpose(tp, logitsT[:, t * P : (t + 1) * P], ident)
            nc.vector.tensor_copy(out=logits3[:, t, :], in_=tp)

        for ti in range(ntp):
            t = nt0 + ti
            lg = logits3[:, t, :]
            mx8 = small.tile([P, 8], F32, tag="mx8")
            nc.vector.max(out=mx8, in_=lg)
            ix8 = small.tile([P, 8], U32, tag="ix8")
            nc.vector.max_index(out=ix8, in_max=mx8, in_values=lg)
            nc.vector.tensor_copy(out=idxacc[:, t, :], in_=ix8[:, 0:2])
            nc.vector.tensor_scalar_mul(negmx3[:, t : t + 1], mx8[:, 0:1], -1.0)

    def stats_post(ph):
        # ACT exps + DVE recips + index emission for a completed phase
        tbs, tb0 = PHASE_TBS[ph], PHASE_TB0[ph]
        ntp = tbs * GF // P
        nt0 = tb0 * GF // P
        for ti in range(ntp):
            t = nt0 + ti
            ssum = small.tile([P, 1], F32, tag="ssum")
            nc.scalar.activation(
                out=ex3[:, t, :],
                in_=logits3[:, t, :],
                func=mybir.ActivationFunctionType.Exp,
                bias=negmx3[:, t : t + 1],
                scale=1.0,
                accum_out=ssum,
            )
            nc.vector.reciprocal(out=rec[:, t : t + 1], in_=ssum)
        # Top-2 indices: SBUF [p, i, j] -> DRAM [(i p), j] on the ACT ring
        nc.scalar.dma_start(
            top_idx[nt0 * P : (nt0 + ntp) * P, :].rearrange(
                "(i p) j -> p i j", p=P
            ),
            idxacc[:, nt0 : nt0 + ntp, :],
        )

    n_ph = len(PHASE_TBS)
    for ph in range(n_ph):
        gemm_phase(ph)
        if ph > 0:
            stats_post(ph - 1)
        stats_pre(ph)
    stats_post(n_ph - 1)

    # P_i partials: one PSUM row accumulated by tiny PE matmuls at the end
    # (these wait on the full stats chain, so they must sit after all GEMM
    # matmuls in PE program order)
    for t in range(NT):
        nc.tensor.matmul(
            pp,
            lhsT=rec[:, t : t + 1],
            rhs=ex3[:, t, :],
            start=(t == 0),
            stop=(t == NT - 1),
            skip_group_check=True,
        )

    p_sb = small.tile([1, E], F32, tag="pout")
    nc.vector.tensor_copy(out=p_sb, in_=pp)
    nc.scalar.dma_start(p_part, p_sb)


_COMPILED_NC = None


def _get_compiled():
    global _COMPILED_NC
    if _COMPILED_NC is not None:
        return _COMPILED_NC
    from contextlib import ExitStack

    nc = bacc.Bacc(
        "TRN2",
        target_bir_lowering=False,
        debug=False,
        enable_asserts=False,
        num_devices=N_CORES,
    )
    xt = nc.dram_tensor("xt", [D, T], F32, kind="ExternalInput").ap()
    w = nc.dram_tensor("w", [P, KT * E], F32, kind="ExternalInput").ap()
    b = nc.dram_tensor("b", [E], F32, kind="ExternalInput").ap()
    top_idx = nc.dram_tensor("top_idx", [T, 2], U32, kind="ExternalOutput").ap()
    p_part = nc.dram_tensor("p_part", [1, E], F32, kind="ExternalOutput").ap()

    with tile.TileContext(nc) as tc:
        with ExitStack() as ctx:
            _build_kernel_body(ctx, tc, xt, w, b, top_idx, p_part)
    nc.compile()
    _COMPILED_NC = nc
    return nc


def _run_device(x, W, b, trace=False):
    nc = _get_compiled()
    xf = np.ascontiguousarray(np.asarray(x, dtype=np.float32)).reshape(B * S, D)
    Wf = np.asarray(W, dtype=np.float32)
    # w_packed[p, k*E+e] = W[k*128+p, e]
    w_packed = np.ascontiguousarray(
        Wf.reshape(KT, P, E).transpose(1, 0, 2).reshape(P, KT * E)
    )
    bf = np.ascontiguousarray(np.asarray(b, dtype=np.float32))
    in_maps = []
    for c in range(N_CORES):
        shard = np.ascontiguousarray(xf[c * T : (c + 1) * T, :].T)
        in_maps.append({"xt": shard, "w": w_packed, "b": bf})
    res = bass_utils.run_bass_kernel_spmd(
        nc, in_maps, core_ids=list(range(N_CORES)), trace=trace
    )
    return res


def kernel(x, W, b):
    res = _run_device(x, W, b, trace=False)
    return _assemble(res.results)


def _assemble(results):
    top_u = np.concatenate(
        [results[c]["top_idx"] for c in range(N_CORES)], axis=0
    )  # [B*S, 2] uint32
    top_idx = top_u.astype(np.int32).reshape(B, S, 2)

    p_sum = np.zeros(E, dtype=np.float64)
    for c in range(N_CORES):
        p_sum += results[c]["p_part"][0].astype(np.float64)
    P_i = p_sum / float(B)
    f_i = np.bincount(top_u.ravel().astype(np.int64), minlength=E).astype(
        np.float64
    ) / float(B)
    aux_loss = np.float32(0.01 * E * np.sum(f_i * P_i))

    routed_experts = np.zeros((B, S, E), dtype=np.float32)
    routed_probs = np.zeros((B, S, E), dtype=np.float32)
    return routed_experts, routed_probs, top_idx, aux_loss
